# revision 36
# baseline (speedup 1.0000x reference)
"""Trainium2 Bass kernel for nn_BottomUpNet (dense_mlp).

Reference computation (per row n of N=8192, fully independent across rows):
    summary = aggregate (broadcast)                   # (1024,)
    for k in 0..15:
        x = [summary, towers[n, k, :]]                # (1088,)
        h = relu(x @ OW1 + Ob1); h = relu(h @ OW2 + Ob2)
        pred_k = sigmoid(h @ OW3 + Ob3)
        m = relu(x @ MW1 + Mb1); m = relu(m @ MW2 + Mb2); m = relu(m @ MW3 + Mb3)
        summary = m
    out[n] = prod_k pred_k

Strategy: data-parallel over N across 8 cores (1024 rows each), weights
replicated.  Activations are feature-major ([feature partition, row free]).

fp8 DoubleRow: all five big matmuls (M1s/O1s summary parts, M2, M3, O2) run
in fp8e4 (e4m3) with MatmulPerfMode.DoubleRow -- the PE processes two
128-deep contraction blocks per pass, 2x the bf16 MAC rate.  Weights are
packed host-side into contraction pairs [4][128, 2, 1024] scaled by 2^12;
activations live in fp8 pair tiles [128, 2(pair), 2(rowblk), 512] with
per-tensor power-of-2 scales (summary 2^8, m1 2^6, m2 2^7, h1 2^6).  The
64-deep tower matmuls stay bf16 with weights pre-scaled by s_act*2^12 so
each PSUM group accumulates in one consistent scale.

PSUM tiles are double-bank [128, 2(rowblk), 512]: the two row-block groups
of each output tile fill adjacent banks (with the stationary weights reused
back-to-back), and ONE scalar-engine epilogue relu(psum*k + bias*s_next)
drains both, halving ACT occupancy so it never backs up the PE's PSUM
rotation.  All stationary operands are full 128-row tiles (tower weights
zero-padded) -- partial-row matmuls force a PE tile-config switch that
stalls the following matmul ~145ns.

Measured: 1.185 ms (vs 2.247 ms bf16 baseline, 1.9x), max rel err 7.2e-3
vs the 2e-2 gate; PE busy ~97% of the kernel span at the fp8 roofline.

Other structure:
  - step 0's summary contribution is rank-1 (broadcast aggregate): v =
    agg @ W1s is precomputed exactly on the host (1-row matvec over inputs
    only) and folded into dedicated k=0 epilogue bias columns, so k=0's
    layer 1 is just the tower matmuls; mw1s/ow1s then load dead last,
    never gating the PE start.
  - layer-1 tower matmuls for the M/O branches pair into disjoint PE row
    groups (0-63 / 64-127) so they stream concurrently.
  - the 1024->1 output head is a DVE per-partition multiply/add tree over
    bf16 h2 plus a ones-vector matmul for the cross-partition reduce; its
    sigmoid + product-accumulate are deferred into the next step.  The
    final step's head instead runs as 8 accumulating w3-column matmuls on
    the by-then-idle PE, cutting the tail latency.
  - the final step's M branch is dead (scan carry discarded) and skipped.
"""

import numpy as np
import ml_dtypes

import concourse.bacc as bacc
import concourse.mybir as mybir
import concourse.tile as tile
from concourse.bass import ts, ds
from concourse.bass_utils import run_bass_kernel_spmd

BF16 = ml_dtypes.bfloat16
FP8 = ml_dtypes.float8_e4m3

N_CORES = 8
N = 8192
K = 16
NI = 64          # tower features per step
NH = 1024        # hidden width
FT = NH // 128   # feature tiles (8)
NP = FT // 2     # contraction pairs (4)
R = N // N_CORES  # rows per core (1024)
RB = 512         # row block (matmul moving dim / one PSUM bank)
NR = R // RB     # row blocks per core (2)

# power-of-2 quantization scales
WS = 4096.0      # weight scale (2^12); max |w| ~0.031 -> 127 < 240
S_S = 256.0      # summary act scale (2^8); max ~0.16 -> 41
S_M1 = 64.0      # m1 act scale; max ~0.82 -> 52
S_M2 = 128.0     # m2 act scale; max ~0.36 -> 46
S_H1 = 64.0      # h1 act scale; max ~0.82 -> 52
PS_L1 = S_S * WS          # scale of layer-1 PSUM (2^20)

_BUILT = None


def _build():
    nc = bacc.Bacc("TRN2", target_bir_lowering=False, debug=False,
                   num_devices=N_CORES)
    f32 = mybir.dt.float32
    bf = mybir.dt.bfloat16
    f8 = mybir.dt.float8e4
    DR = mybir.MatmulPerfMode.DoubleRow

    towd = nc.declare_dram_parameter("tow", [K, NI, R], bf, isOutput=False)
    mw1sd = nc.declare_dram_parameter("mw1s", [NP, 128, 2, NH], f8, isOutput=False)
    mw1td = nc.declare_dram_parameter("mw1t", [NI, NH], bf, isOutput=False)
    mw2d = nc.declare_dram_parameter("mw2", [NP, 128, 2, NH], f8, isOutput=False)
    mw3d = nc.declare_dram_parameter("mw3", [NP, 128, 2, NH], f8, isOutput=False)
    ow1sd = nc.declare_dram_parameter("ow1s", [NP, 128, 2, NH], f8, isOutput=False)
    ow1td = nc.declare_dram_parameter("ow1t", [NI, NH], bf, isOutput=False)
    ow2d = nc.declare_dram_parameter("ow2", [NP, 128, 2, NH], f8, isOutput=False)
    w3cd = nc.declare_dram_parameter("w3c", [128, FT], f32, isOutput=False)
    w3bd = nc.declare_dram_parameter("w3b", [128, FT], bf, isOutput=False)
    balld = nc.declare_dram_parameter("ball", [128, 56], f32, isOutput=False)
    ob3d = nc.declare_dram_parameter("ob3", [1, 1], f32, isOutput=False)
    outd = nc.declare_dram_parameter("out", [1, R], f32, isOutput=True)

    Relu = mybir.ActivationFunctionType.Relu
    Sigmoid = mybir.ActivationFunctionType.Sigmoid
    Identity = mybir.ActivationFunctionType.Identity
    Add = mybir.AluOpType.add
    Mult = mybir.AluOpType.mult

    # epilogue scale constants: out_next = relu(psum * k + b * s_next)
    K_M1 = S_M1 / PS_L1
    K_O1 = S_H1 / PS_L1
    K_M2 = S_M2 / (S_M1 * WS)
    K_M3 = S_S / (S_M2 * WS)
    K_O2 = 1.0 / (S_H1 * WS)   # h2 stored in true units (bf16)

    with tile.TileContext(nc) as tc:
        with (
            tc.tile_pool(name="weights", bufs=1) as wp,
            tc.tile_pool(name="summary", bufs=1) as sp,
            tc.tile_pool(name="acts", bufs=4) as ap,
            tc.tile_pool(name="tow", bufs=4) as twp,
            tc.tile_pool(name="small", bufs=1) as smp,
            tc.tile_pool(name="zwork", bufs=2) as zw,
            tc.tile_pool(name="psum", bufs=3, space="PSUM") as pp,
            tc.tile_pool(name="zpsum", bufs=2, space="PSUM") as zp,
        ):
            def load_w_split(dram, name, engs):
                tiles = []
                for i in range(NP):
                    t = wp.tile([128, 2, NH], f8, tag=f"{name}{i}",
                                name=f"{name}{i}")
                    engs[i % len(engs)].dma_start(out=t, in_=dram[i])
                    tiles.append(t)
                return tiles

            # The ACT sequencer issues NO DMAs: a dma_start on a sequencer
            # blocks it for the transfer, and ACT runs every epilogue --
            # k0's first epilogues must not sit behind weight loads.  Big
            # weights + the per-step tower stream ride the sync HW queue in
            # strict first-use order; smalls ride the gpsimd SW queue.
            ball = smp.tile([128, 56], f32, tag="ball", name="ball")
            nc.sync.dma_start(out=ball, in_=balld[:])
            tow0 = twp.tile([128, R], bf, tag="tow", name="tow")
            nc.sync.dma_start(out=tow0[0:NI, :], in_=towd[0])
            mw1t = wp.tile([128, NH], bf, tag="mw1t", name="mw1t")
            nc.gpsimd.memset(mw1t[64:128, :], 0.0)
            nc.sync.dma_start(out=mw1t[0:NI, :], in_=mw1td[:])
            nc.sync.dma_start(out=tow0[64:128, :], in_=towd[0])
            ow1t = wp.tile([128, NH], bf, tag="ow1t", name="ow1t")
            nc.gpsimd.memset(ow1t[0:64, :], 0.0)
            nc.sync.dma_start(out=ow1t[64:128, :], in_=ow1td[:])
            ob3 = smp.tile([1, 1], f32, tag="ob3", name="ob3")
            nc.gpsimd.dma_start(out=ob3, in_=ob3d[:])
            w3c = smp.tile([128, FT], f32, tag="w3c", name="w3c")
            nc.gpsimd.dma_start(out=w3c, in_=w3cd[:])
            w3b = smp.tile([128, FT], bf, tag="w3b", name="w3b")
            nc.gpsimd.dma_start(out=w3b, in_=w3bd[:])
            mw2 = load_w_split(mw2d, "mw2", [nc.sync])
            mw3 = load_w_split(mw3d, "mw3", [nc.sync])
            ow2 = load_w_split(ow2d, "ow2", [nc.sync])
            mw1s = load_w_split(mw1sd, "mw1s", [nc.sync])
            ow1s = load_w_split(ow1sd, "ow1s", [nc.sync])

            # ones vector padded to a full 128x128 stationary (col 0 only)
            # so the zjob reduce never switches the PE tile config
            ones = smp.tile([128, 128], bf, tag="ones", name="ones")
            nc.vector.memset(ones, 0.0)
            nc.vector.memset(ones[:, 0:1], 1.0)

            # --- summary double buffer: fp8 pair tiles over both row
            # blocks.  sA is never read at k=0 (step-0 summary contribution
            # is rank-1), so no initialization is needed. ---
            sA = [sp.tile([128, 2, NR, RB], f8, tag=f"sA{i}", name=f"sA{i}")
                  for i in range(NP)]
            sB = [sp.tile([128, 2, NR, RB], f8, tag=f"sB{i}", name=f"sB{i}")
                  for i in range(NP)]

            # --- product accumulators ---
            pacc = []
            for r in range(NR):
                t = smp.tile([1, RB], f32, tag=f"pacc{r}", name=f"pacc{r}")
                nc.vector.memset(t, 1.0)
                pacc.append(t)

            # bias column index per layer: 0=Mb1 1=Mb2 2=Mb3 3=Ob1 4=Ob2
            def epilogue(ot, ps, bias_l, m, k):
                """Single ACT op drains both row-block banks of one m."""
                nc.scalar.activation(ot, ps[:, :, :], Relu,
                                     bias=ball[:, ds(bias_l * 8 + m, 1)],
                                     scale=k)

            def dr_group(ps, ws, rhs, m):
                """Both row-block accumulation groups of output tile m,
                stationary weights back-to-back per contraction pair."""
                for i in range(NP):
                    for r in range(NR):
                        nc.tensor.matmul(
                            ps[:, r, :], ws[i][:, :, ts(m, 128)],
                            rhs[i][:, :, r, :],
                            start=(i == 0), stop=(i == NP - 1),
                            perf_mode=DR)

            def layer1(scur, tow_t, branches=("m", "o")):
                """Fused M/O layer 1.  Per (branch, m): 2x4 DoubleRow fp8
                matmuls over the summary pairs, closed by bf16 tower
                matmuls (M on PE rows 0-63, O on rows 64-127 so each M/O
                pair streams concurrently)."""
                m1o, h1o = [None] * FT, [None] * FT
                for m in range(FT):
                    psm = pso = None
                    if "m" in branches:
                        psm = pp.tile([128, NR, RB], f32, tag="ps",
                                      name="psm")
                        for i in range(NP):
                            for r in range(NR):
                                nc.tensor.matmul(
                                    psm[:, r, :], mw1s[i][:, :, ts(m, 128)],
                                    scur[i][:, :, r, :],
                                    start=(i == 0), stop=False,
                                    perf_mode=DR)
                    if "o" in branches:
                        pso = pp.tile([128, NR, RB], f32, tag="ps",
                                      name="pso")
                        for i in range(NP):
                            for r in range(NR):
                                nc.tensor.matmul(
                                    pso[:, r, :], ow1s[i][:, :, ts(m, 128)],
                                    scur[i][:, :, r, :],
                                    start=(i == 0), stop=False,
                                    perf_mode=DR)
                    for r in range(NR):
                        if "m" in branches:
                            nc.tensor.matmul(
                                psm[:, r, :], mw1t[:, ts(m, 128)],
                                tow_t[:, ts(r, RB)],
                                start=False, stop=True)
                        if "o" in branches:
                            nc.tensor.matmul(
                                pso[:, r, :], ow1t[:, ts(m, 128)],
                                tow_t[:, ts(r, RB)],
                                start=False, stop=True)
                    if "m" in branches:
                        if m % 2 == 0:
                            mt = ap.tile([128, 2, NR, RB], f8, tag="m1",
                                         name="m1")
                            m1o[m // 2] = mt
                        epilogue(mt[:, m % 2, :, :], psm, 0, m, K_M1)
                    if "o" in branches:
                        if m % 2 == 0:
                            ht = ap.tile([128, 2, NR, RB], f8, tag="h1",
                                         name="h1")
                            h1o[m // 2] = ht
                        epilogue(ht[:, m % 2, :, :], pso, 3, m, K_O1)
                return m1o[:NP], h1o[:NP]

            def layer(rhs, ws, bias_l, k, out_mode, out_tiles=None):
                """rhs: [NP] fp8 pair tiles.  out_mode: 'pair' -> new fp8
                pair tiles, 'spair' -> write into out_tiles (summary
                pairs), 'flat' -> bf16 flat tiles (h2, both row blocks)."""
                outs = [None] * FT
                cur = None
                for m in range(FT):
                    ps = pp.tile([128, NR, RB], f32, tag="ps", name="ps")
                    dr_group(ps, ws, rhs, m)
                    if out_mode == "flat":
                        # h2 feeds only the head, so it is stored UNSCALED
                        # (psum units); 1/(S_H1*WS) is folded into the w3
                        # columns host-side.  Bias-only => one DVE op,
                        # keeping ACT free for the fp8 epilogues.
                        ot = ap.tile([128, R], bf, tag="h2", name="h2",
                                     bufs=8)
                        nc.vector.tensor_scalar(
                            ot[:], ps[:, :, :],
                            ball[:, ds(bias_l * 8 + m, 1)], 0.0, Add,
                            mybir.AluOpType.max)
                        outs[m] = ot
                    else:
                        if out_mode == "spair":
                            cur = out_tiles[m // 2]
                        elif m % 2 == 0:
                            cur = ap.tile([128, 2, NR, RB], f8, tag="l2",
                                          name="l2")
                        epilogue(cur[:, m % 2, :, :], ps, bias_l, m, k)
                        outs[m] = cur
                if out_mode == "flat":
                    return outs
                return [outs[2 * p] for p in range(NP)]

            # Step-0 rank-1 trick: summary0 = broadcast(aggregate) is the
            # same for every row, so its layer-1 contribution v = agg @ W1s
            # is a per-partition CONSTANT per output tile -- precomputed
            # exactly on the host and folded straight into the k=0 epilogue
            # bias columns (ball cols 40-55).  k=0 layer 1 is then just the
            # tower matmuls.
            def layer1_k0(tow_t):
                m1o, h1o = [None] * FT, [None] * FT
                for br in ("m", "o"):
                    for m in range(FT):
                        ps = pp.tile([128, NR, RB], f32, tag="ps",
                                     name="psk0")
                        for r in range(NR):
                            if br == "m":
                                nc.tensor.matmul(
                                    ps[:, r, :], mw1t[:, ts(m, 128)],
                                    tow_t[:, ts(r, RB)],
                                    start=True, stop=True)
                            else:
                                nc.tensor.matmul(
                                    ps[:, r, :], ow1t[:, ts(m, 128)],
                                    tow_t[:, ts(r, RB)],
                                    start=True, stop=True)
                        if br == "m":
                            if m % 2 == 0:
                                mt = ap.tile([128, 2, NR, RB], f8,
                                             tag="m1", name="m1")
                                m1o[m // 2] = mt
                            epilogue(mt[:, m % 2, :, :], ps, 5, m, K_M1)
                        else:
                            if m % 2 == 0:
                                ht = ap.tile([128, 2, NR, RB], f8,
                                             tag="h1", name="h1")
                                h1o[m // 2] = ht
                            epilogue(ht[:, m % 2, :, :], ps, 6, m, K_O1)
                return m1o[:NP], h1o[:NP]

            def flush_zjobs(zjobs):
                for gb, r in zjobs:
                    zps = zp.tile([128, RB], f32, tag="z", name="zps")
                    nc.tensor.matmul(zps[:], ones[:], gb[:, ts(r, RB)],
                                     start=True, stop=True)
                    pr = smp.tile([1, RB], f32, tag=f"pr{r}",
                                  name=f"pr{r}")
                    nc.scalar.activation(pr[:], zps[0:1, :], Sigmoid,
                                         bias=ob3[:])
                    nc.vector.tensor_mul(pacc[r][:], pacc[r][:], pr[:])

            scur, snxt = sA, sB
            zjobs = []
            for k in range(K):
                if k == 0:
                    tow_t = tow0
                else:
                    tow_t = twp.tile([128, R], bf, tag="tow", name="tow")
                    nc.sync.dma_start(out=tow_t[0:NI, :], in_=towd[k])
                    nc.sync.dma_start(out=tow_t[64:128, :], in_=towd[k])

                if k == 0:
                    m1, h1 = layer1_k0(tow_t)
                elif k == K - 1:
                    # the final scan carry is discarded by the reference, so
                    # the last step's M branch (M1/M2/M3) is dead code
                    m1, h1 = layer1(scur, tow_t, branches=("o",))
                else:
                    m1, h1 = layer1(scur, tow_t)
                if k < K - 1:
                    m2 = layer(m1, mw2, 1, K_M2, "pair")
                    # previous step's output head (its DVE reduce is long
                    # done, so the sigmoid never head-of-line-blocks ACT)
                    flush_zjobs(zjobs)
                    zjobs = []
                    layer(m2, mw3, 2, K_M3, "spair", out_tiles=snxt)
                else:
                    flush_zjobs(zjobs)
                    zjobs = []
                h2 = layer(h1, ow2, 4, K_O2, "flat")
                # g = sum_i h2_i * w3_i on the DVE (per-partition scalars),
                # reduced across partitions next step by a ones-matmul.
                if k < K - 1:
                    # DVE-serial chain over both row blocks; latency is
                    # hidden by the next step's PE work
                    g = zw.tile([128, R], f32, tag="g", name="g")
                    nc.vector.tensor_scalar(
                        g[:], h2[0][:], w3c[:, ds(0, 1)], None, Mult)
                    for i in range(1, FT):
                        t = zw.tile([128, R], f32, tag="t", name="t",
                                    bufs=3)
                        nc.vector.tensor_scalar(
                            t[:], h2[i][:], w3c[:, ds(i, 1)], None, Mult)
                        nc.vector.tensor_tensor(g[:], g[:], t[:], Add)
                    gb = zw.tile([128, R], bf, tag="gb", name="gb", bufs=2)
                    nc.vector.tensor_copy(gb[:], g[:])
                    zjobs.append((gb, 0))
                    zjobs.append((gb, 1))
                else:
                    # final step: the PE is idle by now, so the whole
                    # 1024->1 reduce runs as 8 accumulating matmuls (bf16
                    # w3 columns as stationary) straight into [1, RB]
                    # PSUMs, skipping the DVE chain on the critical tail.
                    for r in range(NR):
                        zps = zp.tile([1, RB], f32, tag="z", name="zps")
                        for i in range(FT):
                            nc.tensor.matmul(
                                zps[:], w3b[:, ds(i, 1)],
                                h2[i][:, ts(r, RB)],
                                start=(i == 0), stop=(i == FT - 1))
                        pr = smp.tile([1, RB], f32, tag=f"pr{r}",
                                      name=f"pr{r}")
                        nc.scalar.activation(pr[:], zps[:], Sigmoid,
                                             bias=ob3[:])
                        nc.vector.tensor_mul(pacc[r][:], pacc[r][:],
                                             pr[:])

                scur, snxt = snxt, scur

            for r in range(NR):
                nc.sync.dma_start(out=outd[:, ts(r, RB)], in_=pacc[r][:])

    nc.finalize()
    return nc


def _get_nc():
    global _BUILT
    if _BUILT is None:
        _BUILT = _build()
    return _BUILT


def _pack_pairs(W, scale):
    """[1024, NH] f32 -> [NP, 128, 2, NH] fp8 contraction pairs."""
    Wq = (np.asarray(W, np.float32) * scale).astype(FP8)
    return np.ascontiguousarray(
        Wq.reshape(NP, 2, 128, NH).transpose(0, 2, 1, 3))


def _prep_inputs(inputs):
    f32 = np.float32
    towers = np.asarray(inputs["towers"], dtype=f32)
    agg = np.asarray(inputs["aggregate"], dtype=f32)
    MW1 = np.asarray(inputs["MW1"], dtype=f32)
    OW1 = np.asarray(inputs["OW1"], dtype=f32)

    biases = []
    for bname, s in (("Mb1", S_M1), ("Mb2", S_M2), ("Mb3", S_S),
                     ("Ob1", S_H1), ("Ob2", S_H1 * WS)):
        biases.append((np.asarray(inputs[bname], f32) * s).reshape(FT, 128).T)

    # step-0 rank-1 layer-1 contribution, exact on host (1-row matvec),
    # folded into the k=0 epilogue bias columns
    v_m = (agg.reshape(NH) @ MW1[:NH])
    v_o = (agg.reshape(NH) @ OW1[:NH])
    biases.append(((np.asarray(inputs["Mb1"], f32) + v_m) * S_M1)
                  .reshape(FT, 128).T)
    biases.append(((np.asarray(inputs["Ob1"], f32) + v_o) * S_H1)
                  .reshape(FT, 128).T)

    shared = {
        "mw1s": _pack_pairs(MW1[:NH], WS),
        "mw1t": np.ascontiguousarray(MW1[NH:] * PS_L1).astype(BF16),
        "mw2": _pack_pairs(inputs["MW2"], WS),
        "mw3": _pack_pairs(inputs["MW3"], WS),
        "ow1s": _pack_pairs(OW1[:NH], WS),
        "ow1t": np.ascontiguousarray(OW1[NH:] * PS_L1).astype(BF16),
        "ow2": _pack_pairs(inputs["OW2"], WS),
        "w3c": np.ascontiguousarray(
            np.asarray(inputs["OW3"], f32).reshape(FT, 128).T / (S_H1 * WS)),
        "w3b": np.ascontiguousarray(
            np.asarray(inputs["OW3"], f32).reshape(FT, 128).T
            / (S_H1 * WS)).astype(BF16),
        "ball": np.ascontiguousarray(np.concatenate(biases, axis=1)),
        "ob3": np.asarray(inputs["Ob3"], f32).reshape(1, 1),
    }
    in_maps = []
    for c in range(N_CORES):
        tc_ = towers[c * R:(c + 1) * R]          # (R, K, NI)
        towT = np.ascontiguousarray(tc_.transpose(1, 2, 0)).astype(BF16)
        in_maps.append({"tow": towT, **shared})
    return in_maps


def _run(inputs, trace=False):
    nc = _get_nc()
    in_maps = _prep_inputs(inputs)
    res = run_bass_kernel_spmd(nc, in_maps, list(range(N_CORES)), trace=trace)
    out = np.concatenate([res.results[c]["out"][0] for c in range(N_CORES)])
    return out.astype(np.float32), res


def kernel(**inputs):
    out, _ = _run(inputs, trace=False)
    return out
